# revision 10
# baseline (speedup 1.0000x reference)
"""Trainium2 Bass kernel: 3-head routed cross-entropy (moe_routing).

Math (per sample b):
    logits3[b] = hidden_state[b] @ stack(w1,w2,w3).T + stack(b1,b2,b3)   # [3, 10]
    logits[b]  = logits3[b, groups[b]]                                   # [10]
    loss       = mean_b( logsumexp(logits[b]) - logits[b, labels[b]] )

Distribution: data-parallel over 8 NeuronCores, 4096 rows each.

Device layout is class-major (transposed): host packs per-core
hsT = hidden_state_shard.T  [768, 4096], so the PE matmul
    psum[32, 512] += Wt_chunk[128, 32].T @ hsT_chunk[128, 512]
needs no on-device transposes and streams at 1 col/cycle.
All 30 (g,l) class pairs are computed for every sample (3x FLOP padding is
free; the kernel is HBM-bound), then:
  - picked logit sum:  sum_b logits[sel_b, b]  via mask-multiply + reduce (DVE)
  - sumexp per group:  indicator matmul [32->3] on exp(logits+bias) (PE)
  - group-select of ln(sumexp) via one-hot mask reduce (DVE)
Per-core partial sums return to the host, which finishes the mean (the
"all-reduce" of the sharding hint) in float64.
"""

import sys

if "/opt/trn_rl_repo" not in sys.path:
    sys.path.insert(0, "/opt/trn_rl_repo")

import ml_dtypes
import numpy as np

import concourse.bass as bass
import concourse.mybir as mybir
import concourse.tile as tile
from concourse import bacc, bass_utils

B, H, L = 32768, 768, 10
NCORES = 8
BC = B // NCORES          # rows per core
CH = 512                  # batch columns per chunk (one PSUM bank)
NCH = BC // CH            # chunks per core
M = 32                    # class rows, padded 30 -> 32
P = 128                   # SBUF partitions
KC = H // P               # contraction chunks

USE_BF16 = True           # hidden_state/W in bf16 (halves HBM traffic); psum f32


def _dtypes():
    if USE_BF16:
        return mybir.dt.bfloat16, ml_dtypes.bfloat16
    return mybir.dt.float32, np.float32


def _build_program():
    dt_mm, _ = _dtypes()
    f32 = mybir.dt.float32
    nc = bacc.Bacc(trn_type="TRN2", debug=False, num_devices=NCORES)

    hsT = nc.dram_tensor("hsT", [H, BC], dt_mm, kind="ExternalInput").ap()
    wt = nc.dram_tensor("wt", [H, M], dt_mm, kind="ExternalInput").ap()
    biasd = nc.dram_tensor("bias", [M, 1], f32, kind="ExternalInput").ap()
    indd = nc.dram_tensor("ind", [M, 3], f32, kind="ExternalInput").ap()
    seld = nc.dram_tensor("selT", [M, BC], f32, kind="ExternalInput").ap()
    oh3d = nc.dram_tensor("oh3T", [3, BC], f32, kind="ExternalInput").ap()
    out_ln = nc.dram_tensor("out_ln", [3, NCH], f32, kind="ExternalOutput").ap()
    out_pick = nc.dram_tensor("out_pick", [M, NCH], f32, kind="ExternalOutput").ap()

    hsT_r = hsT.rearrange("(k p) n -> p k n", p=P)   # [128, KC, BC]
    wt_r = wt.rearrange("(k p) m -> p k m", p=P)     # [128, KC, M]

    mult = mybir.AluOpType.mult
    add = mybir.AluOpType.add
    Exp = mybir.ActivationFunctionType.Exp
    Ln = mybir.ActivationFunctionType.Ln

    with tile.TileContext(nc) as tc:
        with (
            tc.tile_pool(name="consts", bufs=1) as consts,
            tc.tile_pool(name="hs", bufs=3) as hpool,
            tc.tile_pool(name="work", bufs=3) as wpool,
            tc.tile_pool(name="psL", bufs=2, space="PSUM") as psL,
            tc.tile_pool(name="psG", bufs=2, space="PSUM") as psG,
        ):
            wt_sb = consts.tile([P, KC, M], dt_mm)
            nc.sync.dma_start(out=wt_sb, in_=wt_r)
            bias_sb = consts.tile([M, 1], f32)
            nc.sync.dma_start(out=bias_sb, in_=biasd)
            ind_sb = consts.tile([M, 3], f32)
            nc.sync.dma_start(out=ind_sb, in_=indd)
            sel_sb = consts.tile([M, BC], f32)
            nc.sync.dma_start(out=sel_sb, in_=seld)
            oh3_sb = consts.tile([3, BC], f32)
            nc.sync.dma_start(out=oh3_sb, in_=oh3d)
            acc_pick = consts.tile([M, NCH], f32)
            acc_ln = consts.tile([3, NCH], f32)

            for ch in range(NCH):
                cs = slice(ch * CH, (ch + 1) * CH)
                hs_sb = hpool.tile([P, KC, CH], dt_mm)
                nc.sync.dma_start(out=hs_sb, in_=hsT_r[:, :, cs])

                ps = psL.tile([M, CH], f32)
                for kc in range(KC):
                    nc.tensor.matmul(
                        ps,
                        wt_sb[:, kc, :],
                        hs_sb[:, kc, :],
                        start=(kc == 0),
                        stop=(kc == KC - 1),
                    )

                # sum_b (logits + bias)[sel_b, b] for this chunk
                junkp = wpool.tile([M, CH], f32, tag="junkp")
                nc.vector.scalar_tensor_tensor(
                    out=junkp,
                    in0=ps,
                    scalar=bias_sb[:, :],
                    in1=sel_sb[:, cs],
                    op0=add,
                    op1=mult,
                    accum_out=acc_pick[:, ch : ch + 1],
                )

                # exp(logits + bias)
                ex = wpool.tile([M, CH], f32, tag="exp")
                nc.scalar.activation(out=ex, in_=ps, func=Exp, bias=bias_sb[:, :], scale=1.0)

                # per-group sumexp: [32 -> 3]
                psg = psG.tile([3, CH], f32)
                nc.tensor.matmul(psg, ind_sb[:, :], ex, start=True, stop=True)

                lnt = wpool.tile([3, CH], f32, tag="ln")
                nc.scalar.activation(out=lnt, in_=psg, func=Ln)

                junkl = wpool.tile([3, CH], f32, tag="junkl")
                nc.vector.scalar_tensor_tensor(
                    out=junkl,
                    in0=lnt,
                    scalar=0.0,
                    in1=oh3_sb[:, cs],
                    op0=add,
                    op1=mult,
                    accum_out=acc_ln[:, ch : ch + 1],
                )

            nc.sync.dma_start(out=out_pick, in_=acc_pick)
            nc.sync.dma_start(out=out_ln, in_=acc_ln)

    nc.finalize()
    return nc


def _pack_inputs(hidden_state, w1, b1, w2, b2, w3, b3, groups, labels):
    _, dt_np = _dtypes()
    hs = np.asarray(hidden_state, dtype=np.float32)
    Wpad = np.zeros((M, H), dtype=np.float32)
    Wpad[0:L] = np.asarray(w1, dtype=np.float32)
    Wpad[L : 2 * L] = np.asarray(w2, dtype=np.float32)
    Wpad[2 * L : 3 * L] = np.asarray(w3, dtype=np.float32)
    Wt = np.ascontiguousarray(Wpad.T.astype(dt_np))  # [H, M]

    bias_arr = np.zeros((M, 1), dtype=np.float32)
    bias_arr[0:L, 0] = np.asarray(b1, dtype=np.float32)
    bias_arr[L : 2 * L, 0] = np.asarray(b2, dtype=np.float32)
    bias_arr[2 * L : 3 * L, 0] = np.asarray(b3, dtype=np.float32)

    ind = np.zeros((M, 3), dtype=np.float32)
    for g in range(3):
        ind[g * L : (g + 1) * L, g] = 1.0

    groups = np.asarray(groups).astype(np.int64)
    labels = np.asarray(labels).astype(np.int64)
    col = groups * L + labels                      # [B] in [0, 30)
    rows = np.arange(B)
    selT = np.zeros((M, B), dtype=np.float32)
    selT[col, rows] = 1.0
    oh3 = np.zeros((3, B), dtype=np.float32)
    oh3[groups, rows] = 1.0

    hs_cast = hs.astype(dt_np)
    in_maps = []
    for c in range(NCORES):
        sl = slice(c * BC, (c + 1) * BC)
        in_maps.append(
            {
                "hsT": np.ascontiguousarray(hs_cast[sl].T),
                "wt": Wt,
                "bias": bias_arr,
                "ind": ind,
                "selT": np.ascontiguousarray(selT[:, sl]),
                "oh3T": np.ascontiguousarray(oh3[:, sl]),
            }
        )
    return in_maps


def _run(inputs, trace=False, **kw):
    nc = _build_program()
    in_maps = _pack_inputs(**inputs)
    res = bass_utils.run_bass_kernel_spmd(
        nc, in_maps, list(range(NCORES)), trace=trace, **kw
    )
    total_ln = 0.0
    total_pick = 0.0
    for out in res.results:
        total_ln += float(np.sum(out["out_ln"].astype(np.float64)))
        total_pick += float(np.sum(out["out_pick"].astype(np.float64)))
    loss = (total_ln - total_pick) / B
    return np.float32(loss), res


def kernel(**inputs) -> np.ndarray:
    out, _ = _run(inputs, trace=False)
    return out


def benchmark(inputs, trace=True, **kw):
    """Returns (loss, BassKernelResults) with profiling enabled."""
    return _run(inputs, trace=trace, **kw)


# revision 14
# speedup vs baseline: 1.0858x; 1.0858x over previous
"""Trainium2 Bass kernel: 3-head routed cross-entropy (moe_routing).

Math (per sample b):
    logits3[b] = hidden_state[b] @ stack(w1,w2,w3).T + stack(b1,b2,b3)   # [3, 10]
    logits[b]  = logits3[b, groups[b]]                                   # [10]
    loss       = mean_b( logsumexp(logits[b]) - logits[b, labels[b]] )

Distribution: data-parallel over 8 NeuronCores, 4096 rows each.

Device layout is class-major (transposed): host packs per-core
hsT = hidden_state_shard.T  [768, 4096], so the PE matmul
    psum[32, 512] += Wt_chunk[128, 32].T @ hsT_chunk[128, 512]
needs no on-device transposes and streams at 1 col/cycle.
All 30 (g,l) class pairs are computed for every sample (3x FLOP padding is
free; the kernel is HBM-bound), then:
  - picked logit sum:  sum_b logits[sel_b, b]  via mask-multiply + reduce (DVE)
  - sumexp per group:  indicator matmul [32->3] on exp(logits+bias) (PE)
  - group-select of ln(sumexp) via one-hot mask reduce (DVE)
Per-core partial sums return to the host, which finishes the mean (the
"all-reduce" of the sharding hint) in float64.
"""

import sys

if "/opt/trn_rl_repo" not in sys.path:
    sys.path.insert(0, "/opt/trn_rl_repo")

import ml_dtypes
import numpy as np

import concourse.bass as bass
import concourse.mybir as mybir
import concourse.tile as tile
from concourse import bacc, bass_utils

B, H, L = 32768, 768, 10
NCORES = 8
BC = B // NCORES          # rows per core
CH = 512                  # batch columns per chunk (one PSUM bank)
NCH = BC // CH            # chunks per core
M = 32                    # class rows, padded 30 -> 32
P = 128                   # SBUF partitions
KC = H // P               # contraction chunks

USE_BF16 = True           # hidden_state/W in bf16 (halves HBM traffic); psum f32


def _dtypes():
    if USE_BF16:
        return mybir.dt.bfloat16, ml_dtypes.bfloat16
    return mybir.dt.float32, np.float32


def _build_program():
    dt_mm, _ = _dtypes()
    f32 = mybir.dt.float32
    nc = bacc.Bacc(trn_type="TRN2", debug=False, num_devices=NCORES)

    # hsT packed per chunk, per-partition contiguous: [NCH, P, KC*CH]
    # element (ch, p, kc*CH + n) = hidden_state[ch*CH + n, kc*P + p]
    hsT = nc.dram_tensor("hsT", [NCH, P, KC * CH], dt_mm, kind="ExternalInput").ap()
    wt = nc.dram_tensor("wt", [H, M], dt_mm, kind="ExternalInput").ap()
    biasd = nc.dram_tensor("bias", [M, 1], f32, kind="ExternalInput").ap()
    indd = nc.dram_tensor("ind", [M, 3], f32, kind="ExternalInput").ap()
    seld = nc.dram_tensor("selT", [M, BC], dt_mm, kind="ExternalInput").ap()
    oh3d = nc.dram_tensor("oh3T", [3, BC], dt_mm, kind="ExternalInput").ap()
    out_ln = nc.dram_tensor("out_ln", [3, NCH], f32, kind="ExternalOutput").ap()
    out_pick = nc.dram_tensor("out_pick", [M, NCH], f32, kind="ExternalOutput").ap()

    wt_r = wt.rearrange("(k p) m -> p k m", p=P)     # [128, KC, M]

    mult = mybir.AluOpType.mult
    add = mybir.AluOpType.add
    Exp = mybir.ActivationFunctionType.Exp
    Ln = mybir.ActivationFunctionType.Ln

    with tile.TileContext(nc) as tc:
        with (
            tc.tile_pool(name="consts", bufs=1) as consts,
            tc.tile_pool(name="hs", bufs=3) as hpool,
            tc.tile_pool(name="work", bufs=3) as wpool,
            tc.tile_pool(name="psL", bufs=2, space="PSUM") as psL,
            tc.tile_pool(name="psG", bufs=2, space="PSUM") as psG,
        ):
            wt_sb = consts.tile([P, KC, M], dt_mm)
            nc.sync.dma_start(out=wt_sb, in_=wt_r)
            bias_sb = consts.tile([M, 1], f32)
            nc.sync.dma_start(out=bias_sb, in_=biasd)
            ind_sb = consts.tile([M, 3], f32)
            nc.sync.dma_start(out=ind_sb, in_=indd)
            sel_sb = consts.tile([M, BC], dt_mm)
            nc.sync.dma_start(out=sel_sb, in_=seld)
            oh3_sb = consts.tile([3, BC], dt_mm)
            nc.sync.dma_start(out=oh3_sb, in_=oh3d)
            acc_pick = consts.tile([M, NCH], f32)
            acc_ln = consts.tile([3, NCH], f32)

            for ch in range(NCH):
                cs = slice(ch * CH, (ch + 1) * CH)
                hs_sb = hpool.tile([P, KC, CH], dt_mm)
                nc.sync.dma_start(
                    out=hs_sb, in_=hsT[ch].rearrange("p (k n) -> p k n", k=KC)
                )

                ps = psL.tile([M, CH], f32)
                for kc in range(KC):
                    nc.tensor.matmul(
                        ps,
                        wt_sb[:, kc, :],
                        hs_sb[:, kc, :],
                        start=(kc == 0),
                        stop=(kc == KC - 1),
                    )

                # sum_b (logits + bias)[sel_b, b] for this chunk
                junkp = wpool.tile([M, CH], f32, tag="junkp")
                nc.vector.scalar_tensor_tensor(
                    out=junkp,
                    in0=ps,
                    scalar=bias_sb[:, :],
                    in1=sel_sb[:, cs],
                    op0=add,
                    op1=mult,
                    accum_out=acc_pick[:, ch : ch + 1],
                )

                # exp(logits + bias)
                ex = wpool.tile([M, CH], f32, tag="exp")
                nc.scalar.activation(out=ex, in_=ps, func=Exp, bias=bias_sb[:, :], scale=1.0)

                # per-group sumexp: [32 -> 3]
                psg = psG.tile([3, CH], f32)
                nc.tensor.matmul(psg, ind_sb[:, :], ex, start=True, stop=True)

                lnt = wpool.tile([3, CH], f32, tag="ln")
                nc.scalar.activation(out=lnt, in_=psg, func=Ln)

                junkl = wpool.tile([3, CH], f32, tag="junkl")
                nc.vector.scalar_tensor_tensor(
                    out=junkl,
                    in0=lnt,
                    scalar=0.0,
                    in1=oh3_sb[:, cs],
                    op0=add,
                    op1=mult,
                    accum_out=acc_ln[:, ch : ch + 1],
                )

            nc.sync.dma_start(out=out_pick, in_=acc_pick)
            nc.sync.dma_start(out=out_ln, in_=acc_ln)

    nc.finalize()
    return nc


def _pack_inputs(hidden_state, w1, b1, w2, b2, w3, b3, groups, labels):
    _, dt_np = _dtypes()
    hs = np.asarray(hidden_state, dtype=np.float32)
    Wpad = np.zeros((M, H), dtype=np.float32)
    Wpad[0:L] = np.asarray(w1, dtype=np.float32)
    Wpad[L : 2 * L] = np.asarray(w2, dtype=np.float32)
    Wpad[2 * L : 3 * L] = np.asarray(w3, dtype=np.float32)
    Wt = np.ascontiguousarray(Wpad.T.astype(dt_np))  # [H, M]

    bias_arr = np.zeros((M, 1), dtype=np.float32)
    bias_arr[0:L, 0] = np.asarray(b1, dtype=np.float32)
    bias_arr[L : 2 * L, 0] = np.asarray(b2, dtype=np.float32)
    bias_arr[2 * L : 3 * L, 0] = np.asarray(b3, dtype=np.float32)

    ind = np.zeros((M, 3), dtype=np.float32)
    for g in range(3):
        ind[g * L : (g + 1) * L, g] = 1.0

    groups = np.asarray(groups).astype(np.int64)
    labels = np.asarray(labels).astype(np.int64)
    col = groups * L + labels                      # [B] in [0, 30)
    rows = np.arange(B)
    selT = np.zeros((M, B), dtype=dt_np)
    selT[col, rows] = 1.0
    oh3 = np.zeros((3, B), dtype=dt_np)
    oh3[groups, rows] = 1.0

    hs_cast = hs.astype(dt_np)
    in_maps = []
    for c in range(NCORES):
        sl = slice(c * BC, (c + 1) * BC)
        # [BC, H] -> [NCH, CH, KC, P] -> [NCH, P, KC, CH] -> [NCH, P, KC*CH]
        hsp = (
            hs_cast[sl]
            .reshape(NCH, CH, KC, P)
            .transpose(0, 3, 2, 1)
            .reshape(NCH, P, KC * CH)
        )
        in_maps.append(
            {
                "hsT": np.ascontiguousarray(hsp),
                "wt": Wt,
                "bias": bias_arr,
                "ind": ind,
                "selT": np.ascontiguousarray(selT[:, sl]),
                "oh3T": np.ascontiguousarray(oh3[:, sl]),
            }
        )
    return in_maps


def _run(inputs, trace=False, **kw):
    nc = _build_program()
    in_maps = _pack_inputs(**inputs)
    res = bass_utils.run_bass_kernel_spmd(
        nc, in_maps, list(range(NCORES)), trace=trace, **kw
    )
    total_ln = 0.0
    total_pick = 0.0
    for out in res.results:
        total_ln += float(np.sum(out["out_ln"].astype(np.float64)))
        total_pick += float(np.sum(out["out_pick"].astype(np.float64)))
    loss = (total_ln - total_pick) / B
    return np.float32(loss), res


def kernel(**inputs) -> np.ndarray:
    out, _ = _run(inputs, trace=False)
    return out


def benchmark(inputs, trace=True, **kw):
    """Returns (loss, BassKernelResults) with profiling enabled."""
    return _run(inputs, trace=trace, **kw)


# revision 15
# speedup vs baseline: 1.1480x; 1.0572x over previous
"""Trainium2 Bass kernel: 3-head routed cross-entropy (moe_routing).

Math (per sample b):
    logits3[b] = hidden_state[b] @ stack(w1,w2,w3).T + stack(b1,b2,b3)   # [3, 10]
    logits[b]  = logits3[b, groups[b]]                                   # [10]
    loss       = mean_b( logsumexp(logits[b]) - logits[b, labels[b]] )

Distribution: data-parallel over 8 NeuronCores, 4096 rows each; host
finishes the scalar mean (the all-reduce of the sharding hint) in f64.

Device layout is class-major (transposed): host packs hsT chunks so the
PE matmul  psum[32, 512] += Wt_chunk[128, 32].T @ hsT_chunk[128, 512]
needs no on-device transposes.  Four 512-sample chunks are packed into
one [128, 512] PSUM tile via col-tiling (tile_position=(0, 32q)): the 4
matmul streams run concurrently on different PE column groups, and all
post-matmul work (exp, group-sum matmul, ln, masked reductions) runs on
128-partition-wide tiles, 4x fewer instructions.

Per 2048-sample superchunk:
  - 6x4 col-tiled matmuls -> logits psum [128, 512] (row j+32q = class j
    of chunk q)
  - ACT: exp(logits + bias) -> [128, 512] SBUF
  - PE:  block-diag indicator matmul [128 -> 12] = per-(chunk, group)
    sumexp
  - ACT: ln -> [12, 512]
  - DVE: scalar_tensor_tensor accumulators:
      acc_pick += sum_b (logits + bias) * onehot(sel column)
      acc_ln   += sum_b ln(sumexp) * onehot(group)
Host: loss = (sum(acc_ln) - sum(acc_pick)) / B.
"""

import sys

if "/opt/trn_rl_repo" not in sys.path:
    sys.path.insert(0, "/opt/trn_rl_repo")

import ml_dtypes
import numpy as np

import concourse.bass as bass
import concourse.mybir as mybir
import concourse.tile as tile
from concourse import bacc, bass_utils

B, H, L = 32768, 768, 10
NCORES = 8
BC = B // NCORES          # rows per core
CH = 512                  # batch columns per chunk (PSUM bank / matmul N)
NCH = BC // CH            # chunks per core
Q = 4                     # chunks packed per PSUM tile (col-tiling)
NSC = NCH // Q            # superchunks per core
M = 32                    # class rows, padded 30 -> 32
P = 128                   # SBUF partitions
KC = H // P               # contraction chunks

USE_BF16 = True           # hidden_state/W/masks in bf16; psum/accum f32


def _dtypes():
    if USE_BF16:
        return mybir.dt.bfloat16, ml_dtypes.bfloat16
    return mybir.dt.float32, np.float32


def _build_program():
    dt_mm, _ = _dtypes()
    f32 = mybir.dt.float32
    nc = bacc.Bacc(trn_type="TRN2", debug=False, num_devices=NCORES)

    # hsT packed per chunk, per-partition contiguous: [NCH, P, KC*CH]
    # element (ch, p, kc*CH + n) = hidden_state[ch*CH + n, kc*P + p]
    hsT = nc.dram_tensor("hsT", [NCH, P, KC * CH], dt_mm, kind="ExternalInput").ap()
    wt = nc.dram_tensor("wt", [H, M], dt_mm, kind="ExternalInput").ap()
    # bias4: bias replicated over the 4 chunk slots -> [128, 1]
    biasd = nc.dram_tensor("bias4", [P, 1], f32, kind="ExternalInput").ap()
    # ind4: block-diag [32, 3] group indicator per chunk slot -> [128, 12]
    indd = nc.dram_tensor("ind4", [P, Q * 3], f32, kind="ExternalInput").ap()
    # sel4: [128, NSC*CH]; row j+32q, col sc*CH+n = 1{class j == sel(b)},
    # b = sc*Q*CH + q*CH + n.  oh12: [12, NSC*CH] likewise for groups.
    seld = nc.dram_tensor("sel4", [P, NSC * CH], dt_mm, kind="ExternalInput").ap()
    oh3d = nc.dram_tensor("oh12", [Q * 3, NSC * CH], dt_mm, kind="ExternalInput").ap()
    out_ln = nc.dram_tensor("out_ln", [Q * 3, NSC], f32, kind="ExternalOutput").ap()
    out_pick = nc.dram_tensor("out_pick", [P, NSC], f32, kind="ExternalOutput").ap()

    wt_r = wt.rearrange("(k p) m -> p k m", p=P)     # [128, KC, M]

    add = mybir.AluOpType.add
    mult = mybir.AluOpType.mult
    Exp = mybir.ActivationFunctionType.Exp
    Ln = mybir.ActivationFunctionType.Ln

    with tile.TileContext(nc) as tc:
        with (
            tc.tile_pool(name="consts", bufs=1) as consts,
            tc.tile_pool(name="hs", bufs=2 * Q) as hpool,
            tc.tile_pool(name="work", bufs=3) as wpool,
            tc.tile_pool(name="psL", bufs=2, space="PSUM") as psL,
            tc.tile_pool(name="psG", bufs=2, space="PSUM") as psG,
        ):
            wt_sb = consts.tile([P, KC, M], dt_mm)
            nc.sync.dma_start(out=wt_sb, in_=wt_r)
            bias_sb = consts.tile([P, 1], f32)
            nc.sync.dma_start(out=bias_sb, in_=biasd)
            ind_sb = consts.tile([P, Q * 3], f32)
            nc.sync.dma_start(out=ind_sb, in_=indd)
            sel_sb = consts.tile([P, NSC * CH], dt_mm)
            nc.sync.dma_start(out=sel_sb, in_=seld)
            oh3_sb = consts.tile([Q * 3, NSC * CH], dt_mm)
            nc.sync.dma_start(out=oh3_sb, in_=oh3d)
            acc_pick = consts.tile([P, NSC], f32)
            acc_ln = consts.tile([Q * 3, NSC], f32)

            for sc in range(NSC):
                cs = slice(sc * CH, (sc + 1) * CH)
                hs_q = []
                for q in range(Q):
                    hs_sb = hpool.tile([P, KC, CH], dt_mm, tag="hs")
                    nc.sync.dma_start(
                        out=hs_sb,
                        in_=hsT[sc * Q + q].rearrange("p (k n) -> p k n", k=KC),
                    )
                    hs_q.append(hs_sb)

                ps = psL.tile([P, CH], f32)
                for kc in range(KC):
                    for q in range(Q):
                        nc.tensor.matmul(
                            ps[32 * q : 32 * (q + 1), :],
                            wt_sb[:, kc, :],
                            hs_q[q][:, kc, :],
                            start=(kc == 0),
                            stop=(kc == KC - 1),
                            tile_position=(0, 32 * q),
                        )

                # sum_b (logits + bias)[sel_b, b] for this superchunk
                junkp = wpool.tile([P, CH], f32, tag="junkp")
                nc.vector.scalar_tensor_tensor(
                    out=junkp,
                    in0=ps,
                    scalar=bias_sb[:, :],
                    in1=sel_sb[:, cs],
                    op0=add,
                    op1=mult,
                    accum_out=acc_pick[:, sc : sc + 1],
                )

                # exp(logits + bias)
                ex = wpool.tile([P, CH], f32, tag="exp")
                nc.scalar.activation(out=ex, in_=ps, func=Exp, bias=bias_sb[:, :], scale=1.0)

                # per-(chunk, group) sumexp: [128 -> 12]
                psg = psG.tile([Q * 3, CH], f32)
                nc.tensor.matmul(psg, ind_sb[:, :], ex, start=True, stop=True)

                lnt = wpool.tile([Q * 3, CH], f32, tag="ln")
                nc.scalar.activation(out=lnt, in_=psg, func=Ln)

                junkl = wpool.tile([Q * 3, CH], f32, tag="junkl")
                nc.vector.scalar_tensor_tensor(
                    out=junkl,
                    in0=lnt,
                    scalar=0.0,
                    in1=oh3_sb[:, cs],
                    op0=add,
                    op1=mult,
                    accum_out=acc_ln[:, sc : sc + 1],
                )

            nc.sync.dma_start(out=out_pick, in_=acc_pick)
            nc.sync.dma_start(out=out_ln, in_=acc_ln)

    nc.finalize()
    return nc


def _pack_inputs(hidden_state, w1, b1, w2, b2, w3, b3, groups, labels):
    _, dt_np = _dtypes()
    hs = np.asarray(hidden_state, dtype=np.float32)
    Wpad = np.zeros((M, H), dtype=np.float32)
    Wpad[0:L] = np.asarray(w1, dtype=np.float32)
    Wpad[L : 2 * L] = np.asarray(w2, dtype=np.float32)
    Wpad[2 * L : 3 * L] = np.asarray(w3, dtype=np.float32)
    Wt = np.ascontiguousarray(Wpad.T.astype(dt_np))  # [H, M]

    bias1 = np.zeros(M, dtype=np.float32)
    bias1[0:L] = np.asarray(b1, dtype=np.float32)
    bias1[L : 2 * L] = np.asarray(b2, dtype=np.float32)
    bias1[2 * L : 3 * L] = np.asarray(b3, dtype=np.float32)
    bias4 = np.tile(bias1, Q)[:, None].copy()        # [128, 1]

    ind1 = np.zeros((M, 3), dtype=np.float32)
    for g in range(3):
        ind1[g * L : (g + 1) * L, g] = 1.0
    ind4 = np.zeros((P, Q * 3), dtype=np.float32)    # block diag
    for q in range(Q):
        ind4[q * M : (q + 1) * M, q * 3 : (q + 1) * 3] = ind1

    groups = np.asarray(groups).astype(np.int64)
    labels = np.asarray(labels).astype(np.int64)
    col = groups * L + labels                        # [B] in [0, 30)

    hs_cast = hs.astype(dt_np)
    in_maps = []
    for c in range(NCORES):
        sl = slice(c * BC, (c + 1) * BC)
        # [BC, H] -> [NCH, CH, KC, P] -> [NCH, P, KC, CH] -> [NCH, P, KC*CH]
        hsp = (
            hs_cast[sl]
            .reshape(NCH, CH, KC, P)
            .transpose(0, 3, 2, 1)
            .reshape(NCH, P, KC * CH)
        )
        # masks in packed layout: b = sc*Q*CH + q*CH + n -> row block q, col sc*CH+n
        colc = col[sl].reshape(NSC, Q, CH)
        gc = groups[sl].reshape(NSC, Q, CH)
        n_idx = np.arange(CH)
        sel4 = np.zeros((P, NSC * CH), dtype=dt_np)
        oh12 = np.zeros((Q * 3, NSC * CH), dtype=dt_np)
        for sc in range(NSC):
            for q in range(Q):
                sel4[q * M + colc[sc, q], sc * CH + n_idx] = 1.0
                oh12[q * 3 + gc[sc, q], sc * CH + n_idx] = 1.0
        in_maps.append(
            {
                "hsT": np.ascontiguousarray(hsp),
                "wt": Wt,
                "bias4": bias4,
                "ind4": ind4,
                "sel4": sel4,
                "oh12": oh12,
            }
        )
    return in_maps


def _run(inputs, trace=False, **kw):
    nc = _build_program()
    in_maps = _pack_inputs(**inputs)
    res = bass_utils.run_bass_kernel_spmd(
        nc, in_maps, list(range(NCORES)), trace=trace, **kw
    )
    total_ln = 0.0
    total_pick = 0.0
    for out in res.results:
        total_ln += float(np.sum(out["out_ln"].astype(np.float64)))
        total_pick += float(np.sum(out["out_pick"].astype(np.float64)))
    loss = (total_ln - total_pick) / B
    return np.float32(loss), res


def kernel(**inputs) -> np.ndarray:
    out, _ = _run(inputs, trace=False)
    return out


def benchmark(inputs, trace=True, **kw):
    """Returns (loss, BassKernelResults) with profiling enabled."""
    return _run(inputs, trace=trace, **kw)


# revision 19
# speedup vs baseline: 1.2679x; 1.1044x over previous
"""Trainium2 Bass kernel: 3-head routed cross-entropy (moe_routing).

Math (per sample b):
    logits3[b] = hidden_state[b] @ stack(w1,w2,w3).T + stack(b1,b2,b3)   # [3, 10]
    logits[b]  = logits3[b, groups[b]]                                   # [10]
    loss       = mean_b( logsumexp(logits[b]) - logits[b, labels[b]] )

Distribution: data-parallel over 8 NeuronCores, 4096 rows each; host
finishes the scalar mean (the all-reduce of the sharding hint) in f64.

Device layout is class-major (transposed): host packs hsT chunks so the
PE matmul  psum[32, 512] += Wt_chunk[128, 32].T @ hsT_chunk[128, 512]
needs no on-device transposes.  Four 512-sample chunks are packed into
one [128, 512] PSUM tile via col-tiling (tile_position=(0, 32q)): the 4
matmul streams run concurrently on different PE column groups, and all
post-matmul work (exp, group-sum matmul, ln, masked reductions) runs on
128-partition-wide tiles, 4x fewer instructions.

Per 2048-sample superchunk:
  - 6x4 col-tiled matmuls -> logits psum [128, 512] (row j+32q = class j
    of chunk q)
  - ACT: exp(logits + bias) -> [128, 512] SBUF
  - PE:  block-diag indicator matmul [128 -> 12] = per-(chunk, group)
    sumexp
  - ACT: ln -> [12, 512]
  - DVE: scalar_tensor_tensor accumulators:
      acc_pick += sum_b (logits + bias) * onehot(sel column)
      acc_ln   += sum_b ln(sumexp) * onehot(group)
Host: loss = (sum(acc_ln) - sum(acc_pick)) / B.
"""

import sys

if "/opt/trn_rl_repo" not in sys.path:
    sys.path.insert(0, "/opt/trn_rl_repo")

import ml_dtypes
import numpy as np

import concourse.bass as bass
import concourse.mybir as mybir
import concourse.tile as tile
from concourse import bacc, bass_utils

B, H, L = 32768, 768, 10
NCORES = 8
BC = B // NCORES          # rows per core
CH = 512                  # batch columns per chunk (PSUM bank / matmul N)
NCH = BC // CH            # chunks per core
Q = 4                     # chunks packed per PSUM tile (col-tiling)
NSC = NCH // Q            # superchunks per core
M = 32                    # class rows, padded 30 -> 32
P = 128                   # SBUF partitions
KC = H // P               # contraction chunks

USE_BF16 = True           # hidden_state/W/masks in bf16; psum/accum f32


def _dtypes():
    if USE_BF16:
        return mybir.dt.bfloat16, ml_dtypes.bfloat16
    return mybir.dt.float32, np.float32


def _build_program():
    dt_mm, _ = _dtypes()
    f32 = mybir.dt.float32
    nc = bacc.Bacc(trn_type="TRN2", debug=False, num_devices=NCORES)

    # hsT packed per chunk, per-partition contiguous: [NCH, P, KC*CH]
    # element (ch, p, kc*CH + n) = hidden_state[ch*CH + n, kc*P + p]
    hsT = nc.dram_tensor("hsT", [NCH, P, KC * CH], dt_mm, kind="ExternalInput").ap()
    wt = nc.dram_tensor("wt", [H, M], dt_mm, kind="ExternalInput").ap()
    # bias4: bias replicated over the 4 chunk slots -> [128, 1]
    biasd = nc.dram_tensor("bias4", [P, 1], f32, kind="ExternalInput").ap()
    # ind4: block-diag [32, 3] group indicator per chunk slot -> [128, 12]
    indd = nc.dram_tensor("ind4", [P, Q * 3], dt_mm, kind="ExternalInput").ap()
    # sel4: [128, NSC*CH]; row j+32q, col sc*CH+n = 1{class j == sel(b)},
    # b = sc*Q*CH + q*CH + n.  oh12: [12, NSC*CH] likewise for groups.
    seld = nc.dram_tensor("sel4", [P, NSC * CH], dt_mm, kind="ExternalInput").ap()
    oh3d = nc.dram_tensor("oh12", [Q * 3, NSC * CH], dt_mm, kind="ExternalInput").ap()
    out_ln = nc.dram_tensor("out_ln", [Q * 3, NSC], f32, kind="ExternalOutput").ap()
    out_pick = nc.dram_tensor("out_pick", [P, NSC], f32, kind="ExternalOutput").ap()

    wt_r = wt.rearrange("(k p) m -> p k m", p=P)     # [128, KC, M]

    add = mybir.AluOpType.add
    mult = mybir.AluOpType.mult
    Exp = mybir.ActivationFunctionType.Exp
    Ln = mybir.ActivationFunctionType.Ln

    with tile.TileContext(nc) as tc:
        with (
            tc.tile_pool(name="consts", bufs=1) as consts,
            tc.tile_pool(name="hs", bufs=2 * Q) as hpool,
            tc.tile_pool(name="work", bufs=3) as wpool,
            tc.tile_pool(name="psL", bufs=2, space="PSUM") as psL,
            tc.tile_pool(name="psG", bufs=2, space="PSUM") as psG,
        ):
            wt_sb = consts.tile([P, KC, M], dt_mm)
            nc.sync.dma_start(out=wt_sb, in_=wt_r)
            bias_sb = consts.tile([P, 1], f32)
            nc.sync.dma_start(out=bias_sb, in_=biasd)
            ind_sb = consts.tile([P, Q * 3], dt_mm)
            nc.sync.dma_start(out=ind_sb, in_=indd)
            acc_pick = consts.tile([P, NSC], f32)
            acc_ln = consts.tile([Q * 3, NSC], f32)

            # hs chunk DMAs for superchunk 0 go ahead of the (larger) mask
            # DMAs: the masks are not needed until the first DVE/ACT work.
            hs_sc = []
            for sc in range(NSC):
                hs_q = []
                for q in range(Q):
                    hs_sb = hpool.tile([P, KC, CH], dt_mm, tag="hs")
                    nc.sync.dma_start(
                        out=hs_sb,
                        in_=hsT[sc * Q + q].rearrange("p (k n) -> p k n", k=KC),
                    )
                    hs_q.append(hs_sb)
                hs_sc.append(hs_q)
                if sc == 0:
                    sel_sb = consts.tile([P, NSC * CH], dt_mm)
                    nc.sync.dma_start(out=sel_sb, in_=seld)
                    oh3_sb = consts.tile([Q * 3, NSC * CH], dt_mm)
                    nc.sync.dma_start(out=oh3_sb, in_=oh3d)

            psg_sc = []
            for sc in range(NSC):
                cs = slice(sc * CH, (sc + 1) * CH)
                hs_q = hs_sc[sc]
                ps = psL.tile([P, CH], f32)
                for kc in range(KC):
                    for q in range(Q):
                        nc.tensor.matmul(
                            ps[32 * q : 32 * (q + 1), :],
                            wt_sb[:, kc, :],
                            hs_q[q][:, kc, :],
                            start=(kc == 0),
                            stop=(kc == KC - 1),
                            tile_position=(0, 32 * q),
                        )

                # sum_b (logits + bias)[sel_b, b] for this superchunk
                junkp = wpool.tile([P, CH], f32, tag="junkp")
                nc.vector.scalar_tensor_tensor(
                    out=junkp,
                    in0=ps,
                    scalar=bias_sb[:, :],
                    in1=sel_sb[:, cs],
                    op0=add,
                    op1=mult,
                    accum_out=acc_pick[:, sc : sc + 1],
                )

                # exp(logits + bias), cast to bf16 for the fast indicator matmul
                ex = wpool.tile([P, CH], dt_mm, tag="exp")
                nc.scalar.activation(out=ex, in_=ps, func=Exp, bias=bias_sb[:, :], scale=1.0)

                # per-(chunk, group) sumexp: [128 -> 12]
                psg = psG.tile([Q * 3, CH], f32)
                nc.tensor.matmul(psg, ind_sb[:, :], ex, start=True, stop=True)
                psg_sc.append(psg)

            # ln phase batched after all exp phases: the ACT LUT (exp vs ln)
            # reloads cost ~1.3us each, so avoid alternating functions.
            for sc in range(NSC):
                cs = slice(sc * CH, (sc + 1) * CH)
                lnt = wpool.tile([Q * 3, CH], f32, tag="ln")
                nc.scalar.activation(out=lnt, in_=psg_sc[sc], func=Ln)

                junkl = wpool.tile([Q * 3, CH], f32, tag="junkl")
                nc.vector.scalar_tensor_tensor(
                    out=junkl,
                    in0=lnt,
                    scalar=0.0,
                    in1=oh3_sb[:, cs],
                    op0=add,
                    op1=mult,
                    accum_out=acc_ln[:, sc : sc + 1],
                )

            nc.sync.dma_start(out=out_pick, in_=acc_pick)
            nc.sync.dma_start(out=out_ln, in_=acc_ln)

    nc.finalize()
    return nc


def _pack_inputs(hidden_state, w1, b1, w2, b2, w3, b3, groups, labels):
    _, dt_np = _dtypes()
    hs = np.asarray(hidden_state, dtype=np.float32)
    Wpad = np.zeros((M, H), dtype=np.float32)
    Wpad[0:L] = np.asarray(w1, dtype=np.float32)
    Wpad[L : 2 * L] = np.asarray(w2, dtype=np.float32)
    Wpad[2 * L : 3 * L] = np.asarray(w3, dtype=np.float32)
    Wt = np.ascontiguousarray(Wpad.T.astype(dt_np))  # [H, M]

    bias1 = np.zeros(M, dtype=np.float32)
    bias1[0:L] = np.asarray(b1, dtype=np.float32)
    bias1[L : 2 * L] = np.asarray(b2, dtype=np.float32)
    bias1[2 * L : 3 * L] = np.asarray(b3, dtype=np.float32)
    bias4 = np.tile(bias1, Q)[:, None].copy()        # [128, 1]

    ind1 = np.zeros((M, 3), dtype=dt_np)
    for g in range(3):
        ind1[g * L : (g + 1) * L, g] = 1.0
    ind4 = np.zeros((P, Q * 3), dtype=dt_np)         # block diag
    for q in range(Q):
        ind4[q * M : (q + 1) * M, q * 3 : (q + 1) * 3] = ind1

    groups = np.asarray(groups).astype(np.int64)
    labels = np.asarray(labels).astype(np.int64)
    col = groups * L + labels                        # [B] in [0, 30)

    hs_cast = hs.astype(dt_np)
    in_maps = []
    for c in range(NCORES):
        sl = slice(c * BC, (c + 1) * BC)
        # [BC, H] -> [NCH, CH, KC, P] -> [NCH, P, KC, CH] -> [NCH, P, KC*CH]
        hsp = (
            hs_cast[sl]
            .reshape(NCH, CH, KC, P)
            .transpose(0, 3, 2, 1)
            .reshape(NCH, P, KC * CH)
        )
        # masks in packed layout: b = sc*Q*CH + q*CH + n -> row block q, col sc*CH+n
        colc = col[sl].reshape(NSC, Q, CH)
        gc = groups[sl].reshape(NSC, Q, CH)
        n_idx = np.arange(CH)
        sel4 = np.zeros((P, NSC * CH), dtype=dt_np)
        oh12 = np.zeros((Q * 3, NSC * CH), dtype=dt_np)
        for sc in range(NSC):
            for q in range(Q):
                sel4[q * M + colc[sc, q], sc * CH + n_idx] = 1.0
                oh12[q * 3 + gc[sc, q], sc * CH + n_idx] = 1.0
        in_maps.append(
            {
                "hsT": np.ascontiguousarray(hsp),
                "wt": Wt,
                "bias4": bias4,
                "ind4": ind4,
                "sel4": sel4,
                "oh12": oh12,
            }
        )
    return in_maps


def _run(inputs, trace=False, **kw):
    nc = _build_program()
    in_maps = _pack_inputs(**inputs)
    res = bass_utils.run_bass_kernel_spmd(
        nc, in_maps, list(range(NCORES)), trace=trace, **kw
    )
    total_ln = 0.0
    total_pick = 0.0
    for out in res.results:
        total_ln += float(np.sum(out["out_ln"].astype(np.float64)))
        total_pick += float(np.sum(out["out_pick"].astype(np.float64)))
    loss = (total_ln - total_pick) / B
    return np.float32(loss), res


def kernel(**inputs) -> np.ndarray:
    out, _ = _run(inputs, trace=False)
    return out


def benchmark(inputs, trace=True, **kw):
    """Returns (loss, BassKernelResults) with profiling enabled."""
    return _run(inputs, trace=trace, **kw)


# revision 29
# speedup vs baseline: 1.4645x; 1.1551x over previous
"""Trainium2 Bass kernel: 3-head routed cross-entropy (moe_routing).

Math (per sample b):
    logits3[b] = hidden_state[b] @ stack(w1,w2,w3).T + stack(b1,b2,b3)   # [3, 10]
    logits[b]  = logits3[b, groups[b]]                                   # [10]
    loss       = mean_b( logsumexp(logits[b]) - logits[b, labels[b]] )

Distribution: data-parallel over 8 NeuronCores, 4096 rows each; host
finishes the scalar mean (the all-reduce of the sharding hint) in f64.

Device layout is class-major (transposed): host packs hsT chunks so the
PE matmul  psum[32, 512] += Wt_chunk[128, 32].T @ hsT_chunk[128, 512]
needs no on-device transposes.  Four 512-sample chunks are packed into
one [128, 512] PSUM tile via col-tiling (tile_position=(0, 32q)): the 4
matmul streams run concurrently on different PE column groups, and all
post-matmul work (exp, group-sum matmul, ln, masked reductions) runs on
128-partition-wide tiles, 4x fewer instructions.

Per 2048-sample superchunk:
  - 6x4 col-tiled matmuls -> logits psum [128, 512] (row j+32q = class j
    of chunk q)
  - ACT: exp(logits + bias) -> [128, 512] SBUF
  - PE:  block-diag indicator matmul [128 -> 12] = per-(chunk, group)
    sumexp
  - ACT: ln -> [12, 512]
  - DVE: scalar_tensor_tensor accumulators:
      acc_pick += sum_b (logits + bias) * onehot(sel column)
      acc_ln   += sum_b ln(sumexp) * onehot(group)
Host: loss = (sum(acc_ln) - sum(acc_pick)) / B.
"""

import sys

if "/opt/trn_rl_repo" not in sys.path:
    sys.path.insert(0, "/opt/trn_rl_repo")

import ml_dtypes
import numpy as np

import concourse.bass as bass
import concourse.mybir as mybir
import concourse.tile as tile
from concourse import bacc, bass_utils

B, H, L = 32768, 768, 10
NCORES = 8
BC = B // NCORES          # rows per core
CH = 512                  # batch columns per chunk (PSUM bank / matmul N)
NCH = BC // CH            # chunks per core
Q = 4                     # chunks packed per PSUM tile (col-tiling)
NSC = NCH // Q            # superchunks per core
M = 32                    # class rows, padded 30 -> 32
P = 128                   # SBUF partitions
KC = H // P               # contraction chunks

# hidden_state/W dtype on the wire + PE. fp8 halves HBM traffic vs bf16;
# W is pre-scaled by WSCALE on the host so its values sit in fp8's normal
# range, and the 1/WSCALE is folded into the exp's scale / host reduction.
HS_DTYPE = "fp8"          # "fp8" | "bf16" | "f32"
WSCALE = 32.0

_W_TABLE_PATCHED = False


def _dtypes():
    if HS_DTYPE == "fp8":
        dt = mybir.dt.float8e4
    elif HS_DTYPE == "bf16":
        dt = mybir.dt.bfloat16
    else:
        dt = mybir.dt.float32
    return dt, mybir.dt.np(dt)


def _patch_act_tables():
    """Prefer the LUT set holding BOTH Exp and Ln so the ACT engine loads
    one table instead of thrashing exp<->ln loads (~1.3us each)."""
    global _W_TABLE_PATCHED
    if _W_TABLE_PATCHED:
        return
    import concourse.bacc as bacc_mod

    orig = bacc_mod.get_activation_tables

    def patched(arch):
        t = orig(arch)
        if "natural_log_exp_and_others" not in t:
            return t
        # act_func_set_id is positional: keep dict order, but remove Exp/Ln
        # from every other set so the selection pass resolves both to the
        # combined set (whose LUT really holds both functions).
        exp = mybir.ActivationFunctionType.Exp
        ln = mybir.ActivationFunctionType.Ln
        return {
            k: (v if k == "natural_log_exp_and_others" else v - {exp, ln})
            for k, v in t.items()
        }

    bacc_mod.get_activation_tables = patched
    _W_TABLE_PATCHED = True


def _build_program():
    _patch_act_tables()
    dt_mm, _ = _dtypes()
    bf16 = mybir.dt.bfloat16
    f32 = mybir.dt.float32
    nc = bacc.Bacc(trn_type="TRN2", debug=False, num_devices=NCORES)

    # hsT packed per chunk, per-partition contiguous: [NCH, P, KC*CH]
    # element (ch, p, kc*CH + n) = hidden_state[ch*CH + n, kc*P + p]
    hsT = nc.dram_tensor("hsT", [NCH, P, KC * CH], dt_mm, kind="ExternalInput").ap()
    wt = nc.dram_tensor("wt", [H, M], dt_mm, kind="ExternalInput").ap()
    # bias4: bias replicated over the 4 chunk slots -> [128, 1]; biasW is
    # the same scaled by WSCALE (to pair with the scaled logits in psum)
    biasd = nc.dram_tensor("bias4", [P, 1], f32, kind="ExternalInput").ap()
    biasWd = nc.dram_tensor("biasW", [P, 1], f32, kind="ExternalInput").ap()
    # ind4: block-diag [32, 3] group indicator per chunk slot -> [128, 12]
    indd = nc.dram_tensor("ind4", [P, Q * 3], bf16, kind="ExternalInput").ap()
    # sel4: [128, NSC*CH]; row j+32q, col sc*CH+n = 1{class j == sel(b)},
    # b = sc*Q*CH + q*CH + n.  oh12: [12, NSC*CH] likewise for groups.
    seld = nc.dram_tensor("sel4", [P, NSC * CH], bf16, kind="ExternalInput").ap()
    oh3d = nc.dram_tensor("oh12", [Q * 3, NSC * CH], bf16, kind="ExternalInput").ap()
    out_ln = nc.dram_tensor("out_ln", [Q * 3, NSC], f32, kind="ExternalOutput").ap()
    out_pick = nc.dram_tensor("out_pick", [P, NSC], f32, kind="ExternalOutput").ap()

    wt_r = wt.rearrange("(k p) m -> p k m", p=P)     # [128, KC, M]

    add = mybir.AluOpType.add
    mult = mybir.AluOpType.mult
    Exp = mybir.ActivationFunctionType.Exp
    Ln = mybir.ActivationFunctionType.Ln

    with tile.TileContext(nc) as tc:
        with (
            tc.tile_pool(name="consts", bufs=1) as consts,
            tc.tile_pool(name="hs", bufs=2 * Q) as hpool,
            tc.tile_pool(name="work", bufs=3) as wpool,
            tc.tile_pool(name="psL", bufs=2, space="PSUM") as psL,
            tc.tile_pool(name="psG", bufs=2, space="PSUM") as psG,
        ):
            wt_sb = consts.tile([P, KC, M], dt_mm)
            nc.sync.dma_start(out=wt_sb, in_=wt_r)
            bias_sb = consts.tile([P, 1], f32)
            nc.sync.dma_start(out=bias_sb, in_=biasd)
            biasW_sb = consts.tile([P, 1], f32)
            nc.sync.dma_start(out=biasW_sb, in_=biasWd)
            ind_sb = consts.tile([P, Q * 3], bf16)
            nc.sync.dma_start(out=ind_sb, in_=indd)
            acc_pick = consts.tile([P, NSC], f32)
            acc_ln = consts.tile([Q * 3, NSC], f32)

            # hs chunk DMAs for superchunk 0 go ahead of the (larger) mask
            # DMAs: the masks are not needed until the first DVE/ACT work.
            hs_sc = []
            for sc in range(NSC):
                hs_q = []
                for q in range(Q):
                    hs_sb = hpool.tile([P, KC, CH], dt_mm, tag="hs")
                    nc.sync.dma_start(
                        out=hs_sb,
                        in_=hsT[sc * Q + q].rearrange("p (k n) -> p k n", k=KC),
                    )
                    hs_q.append(hs_sb)
                hs_sc.append(hs_q)
                if sc == 0:
                    sel_sb = consts.tile([P, NSC * CH], bf16)
                    nc.sync.dma_start(out=sel_sb, in_=seld)
                    oh3_sb = consts.tile([Q * 3, NSC * CH], bf16)
                    nc.sync.dma_start(out=oh3_sb, in_=oh3d)

            psg_sc = []
            for sc in range(NSC):
                cs = slice(sc * CH, (sc + 1) * CH)
                hs_q = hs_sc[sc]
                ps = psL.tile([P, CH], f32)
                for kc in range(KC):
                    for q in range(Q):
                        nc.tensor.matmul(
                            ps[32 * q : 32 * (q + 1), :],
                            wt_sb[:, kc, :],
                            hs_q[q][:, kc, :],
                            start=(kc == 0),
                            stop=(kc == KC - 1),
                            tile_position=(0, 32 * q),
                        )

                # sum_b WSCALE*(logits + bias)[sel_b, b] for this superchunk
                junkp = wpool.tile([P, CH], f32, tag="junkp")
                nc.vector.scalar_tensor_tensor(
                    out=junkp,
                    in0=ps,
                    scalar=biasW_sb[:, :],
                    in1=sel_sb[:, cs],
                    op0=add,
                    op1=mult,
                    accum_out=acc_pick[:, sc : sc + 1],
                )

                # exp(logits + bias), cast to bf16 for the fast indicator matmul
                ex = wpool.tile([P, CH], bf16, tag="exp")
                nc.scalar.activation(
                    out=ex, in_=ps, func=Exp, bias=bias_sb[:, :], scale=1.0 / WSCALE
                )

                # per-(chunk, group) sumexp: [128 -> 12]
                psg = psG.tile([Q * 3, CH], f32)
                nc.tensor.matmul(psg, ind_sb[:, :], ex, start=True, stop=True)
                psg_sc.append(psg)

            # ln phase batched after all exp phases: the ACT LUT (exp vs ln)
            # reloads cost ~1.3us each, so avoid alternating functions.
            for sc in range(NSC):
                cs = slice(sc * CH, (sc + 1) * CH)
                lnt = wpool.tile([Q * 3, CH], f32, tag="ln")
                nc.scalar.activation(out=lnt, in_=psg_sc[sc], func=Ln)

                junkl = wpool.tile([Q * 3, CH], f32, tag="junkl")
                nc.vector.scalar_tensor_tensor(
                    out=junkl,
                    in0=lnt,
                    scalar=0.0,
                    in1=oh3_sb[:, cs],
                    op0=add,
                    op1=mult,
                    accum_out=acc_ln[:, sc : sc + 1],
                )

            nc.sync.dma_start(out=out_pick, in_=acc_pick)
            nc.sync.dma_start(out=out_ln, in_=acc_ln)

    nc.finalize()
    return nc


def _pack_inputs(hidden_state, w1, b1, w2, b2, w3, b3, groups, labels):
    _, dt_np = _dtypes()
    bf_np = ml_dtypes.bfloat16
    hs = np.asarray(hidden_state, dtype=np.float32)
    Wpad = np.zeros((M, H), dtype=np.float32)
    Wpad[0:L] = np.asarray(w1, dtype=np.float32)
    Wpad[L : 2 * L] = np.asarray(w2, dtype=np.float32)
    Wpad[2 * L : 3 * L] = np.asarray(w3, dtype=np.float32)
    Wt = np.ascontiguousarray((Wpad.T * WSCALE).astype(dt_np))  # [H, M]

    bias1 = np.zeros(M, dtype=np.float32)
    bias1[0:L] = np.asarray(b1, dtype=np.float32)
    bias1[L : 2 * L] = np.asarray(b2, dtype=np.float32)
    bias1[2 * L : 3 * L] = np.asarray(b3, dtype=np.float32)
    bias4 = np.tile(bias1, Q)[:, None].copy()        # [128, 1]
    biasW = (bias4 * WSCALE).astype(np.float32)

    ind1 = np.zeros((M, 3), dtype=bf_np)
    for g in range(3):
        ind1[g * L : (g + 1) * L, g] = 1.0
    ind4 = np.zeros((P, Q * 3), dtype=bf_np)         # block diag
    for q in range(Q):
        ind4[q * M : (q + 1) * M, q * 3 : (q + 1) * 3] = ind1

    groups = np.asarray(groups).astype(np.int64)
    labels = np.asarray(labels).astype(np.int64)
    col = groups * L + labels                        # [B] in [0, 30)

    hs_cast = hs.astype(dt_np)
    in_maps = []
    for c in range(NCORES):
        sl = slice(c * BC, (c + 1) * BC)
        # [BC, H] -> [NCH, CH, KC, P] -> [NCH, P, KC, CH] -> [NCH, P, KC*CH]
        hsp = (
            hs_cast[sl]
            .reshape(NCH, CH, KC, P)
            .transpose(0, 3, 2, 1)
            .reshape(NCH, P, KC * CH)
        )
        # masks in packed layout: b = sc*Q*CH + q*CH + n -> row block q, col sc*CH+n
        colc = col[sl].reshape(NSC, Q, CH)
        gc = groups[sl].reshape(NSC, Q, CH)
        n_idx = np.arange(CH)
        sel4 = np.zeros((P, NSC * CH), dtype=bf_np)
        oh12 = np.zeros((Q * 3, NSC * CH), dtype=bf_np)
        for sc in range(NSC):
            for q in range(Q):
                sel4[q * M + colc[sc, q], sc * CH + n_idx] = 1.0
                oh12[q * 3 + gc[sc, q], sc * CH + n_idx] = 1.0
        in_maps.append(
            {
                "hsT": np.ascontiguousarray(hsp),
                "wt": Wt,
                "bias4": bias4,
                "biasW": biasW,
                "ind4": ind4,
                "sel4": sel4,
                "oh12": oh12,
            }
        )
    return in_maps


def _run(inputs, trace=False, **kw):
    nc = _build_program()
    in_maps = _pack_inputs(**inputs)
    res = bass_utils.run_bass_kernel_spmd(
        nc, in_maps, list(range(NCORES)), trace=trace, **kw
    )
    total_ln = 0.0
    total_pick = 0.0
    for out in res.results:
        total_ln += float(np.sum(out["out_ln"].astype(np.float64)))
        total_pick += float(np.sum(out["out_pick"].astype(np.float64)))
    loss = (total_ln - total_pick / WSCALE) / B
    return np.float32(loss), res


def kernel(**inputs) -> np.ndarray:
    out, _ = _run(inputs, trace=False)
    return out


def benchmark(inputs, trace=True, **kw):
    """Returns (loss, BassKernelResults) with profiling enabled."""
    return _run(inputs, trace=trace, **kw)


# revision 30
# speedup vs baseline: 1.6469x; 1.1245x over previous
"""Trainium2 Bass kernel: 3-head routed cross-entropy (moe_routing).

Math (per sample b):
    logits3[b] = hidden_state[b] @ stack(w1,w2,w3).T + stack(b1,b2,b3)   # [3, 10]
    logits[b]  = logits3[b, groups[b]]                                   # [10]
    loss       = mean_b( logsumexp(logits[b]) - logits[b, labels[b]] )

Distribution: data-parallel over 8 NeuronCores, 4096 rows each; host
finishes the scalar mean (the all-reduce of the sharding hint) in f64.

Device layout is class-major (transposed): host packs hsT chunks so the
PE matmul  psum[32, 512] += Wt_chunk[128, 32].T @ hsT_chunk[128, 512]
needs no on-device transposes.  Four 512-sample chunks are packed into
one [128, 512] PSUM tile via col-tiling (tile_position=(0, 32q)): the 4
matmul streams run concurrently on different PE column groups, and all
post-matmul work (exp, group-sum matmul, ln, masked reductions) runs on
128-partition-wide tiles, 4x fewer instructions.

Per 2048-sample superchunk:
  - 6x4 col-tiled matmuls -> logits psum [128, 512] (row j+32q = class j
    of chunk q)
  - ACT: exp(logits + bias) -> [128, 512] SBUF
  - PE:  block-diag indicator matmul [128 -> 12] = per-(chunk, group)
    sumexp
  - ACT: ln -> [12, 512]
  - DVE: scalar_tensor_tensor accumulators:
      acc_pick += sum_b (logits + bias) * onehot(sel column)
      acc_ln   += sum_b ln(sumexp) * onehot(group)
Host: loss = (sum(acc_ln) - sum(acc_pick)) / B.
"""

import sys

if "/opt/trn_rl_repo" not in sys.path:
    sys.path.insert(0, "/opt/trn_rl_repo")

import ml_dtypes
import numpy as np

import concourse.bass as bass
import concourse.mybir as mybir
import concourse.tile as tile
from concourse import bacc, bass_utils

B, H, L = 32768, 768, 10
NCORES = 8
BC = B // NCORES          # rows per core
CH = 512                  # batch columns per chunk (PSUM bank / matmul N)
NCH = BC // CH            # chunks per core
Q = 4                     # chunks packed per PSUM tile (col-tiling)
NSC = NCH // Q            # superchunks per core
M = 32                    # class rows, padded 30 -> 32
P = 128                   # SBUF partitions
KC = H // P               # contraction chunks

# hidden_state/W dtype on the wire + PE. fp8 halves HBM traffic vs bf16;
# W is pre-scaled by WSCALE on the host so its values sit in fp8's normal
# range, and the 1/WSCALE is folded into the exp's scale / host reduction.
HS_DTYPE = "fp8"          # "fp8" | "bf16" | "f32"
WSCALE = 32.0

_W_TABLE_PATCHED = False


def _dtypes():
    if HS_DTYPE == "fp8":
        dt = mybir.dt.float8e4
    elif HS_DTYPE == "bf16":
        dt = mybir.dt.bfloat16
    else:
        dt = mybir.dt.float32
    return dt, mybir.dt.np(dt)


def _patch_act_tables():
    """Prefer the LUT set holding BOTH Exp and Ln so the ACT engine loads
    one table instead of thrashing exp<->ln loads (~1.3us each)."""
    global _W_TABLE_PATCHED
    if _W_TABLE_PATCHED:
        return
    import concourse.bacc as bacc_mod

    orig = bacc_mod.get_activation_tables

    def patched(arch):
        t = orig(arch)
        if "natural_log_exp_and_others" not in t:
            return t
        # act_func_set_id is positional: keep dict order, but remove Exp/Ln
        # from every other set so the selection pass resolves both to the
        # combined set (whose LUT really holds both functions).
        exp = mybir.ActivationFunctionType.Exp
        ln = mybir.ActivationFunctionType.Ln
        return {
            k: (v if k == "natural_log_exp_and_others" else v - {exp, ln})
            for k, v in t.items()
        }

    bacc_mod.get_activation_tables = patched
    _W_TABLE_PATCHED = True


def _build_program():
    _patch_act_tables()
    dt_mm, _ = _dtypes()
    bf16 = mybir.dt.bfloat16
    f32 = mybir.dt.float32
    nc = bacc.Bacc(trn_type="TRN2", debug=False, num_devices=NCORES)

    # hsT packed per chunk, per-partition contiguous: [NCH, P, KC*CH]
    # element (ch, p, kc*CH + n) = hidden_state[ch*CH + n, kc*P + p]
    hsT = nc.dram_tensor("hsT", [NCH, P, KC * CH], dt_mm, kind="ExternalInput").ap()
    wt = nc.dram_tensor("wt", [H, M], dt_mm, kind="ExternalInput").ap()
    # bias4: bias replicated over the 4 chunk slots -> [128, 1]; biasW is
    # the same scaled by WSCALE (to pair with the scaled logits in psum)
    biasd = nc.dram_tensor("bias4", [P, 1], f32, kind="ExternalInput").ap()
    biasWd = nc.dram_tensor("biasW", [P, 1], f32, kind="ExternalInput").ap()
    # ind4: block-diag [32, 3] group indicator per chunk slot -> [128, 12]
    indd = nc.dram_tensor("ind4", [P, Q * 3], bf16, kind="ExternalInput").ap()
    # sel4: [128, NSC*CH]; row j+32q, col sc*CH+n = 1{class j == sel(b)},
    # b = sc*Q*CH + q*CH + n.  oh12: [12, NSC*CH] likewise for groups.
    seld = nc.dram_tensor("sel4", [P, NSC * CH], bf16, kind="ExternalInput").ap()
    oh3d = nc.dram_tensor("oh12", [Q * 3, NSC * CH], bf16, kind="ExternalInput").ap()
    out_ln = nc.dram_tensor("out_ln", [Q * 3, NSC], f32, kind="ExternalOutput").ap()
    out_pick = nc.dram_tensor("out_pick", [P, NSC], f32, kind="ExternalOutput").ap()

    wt_r = wt.rearrange("(k p) m -> p k m", p=P)     # [128, KC, M]

    add = mybir.AluOpType.add
    mult = mybir.AluOpType.mult
    Exp = mybir.ActivationFunctionType.Exp
    Ln = mybir.ActivationFunctionType.Ln

    with tile.TileContext(nc) as tc:
        with (
            tc.tile_pool(name="consts", bufs=1) as consts,
            tc.tile_pool(name="hs", bufs=2 * Q) as hpool,
            tc.tile_pool(name="work", bufs=3) as wpool,
            tc.tile_pool(name="psL", bufs=2, space="PSUM") as psL,
            tc.tile_pool(name="psG", bufs=2, space="PSUM") as psG,
        ):
            # hs chunks stream on the Sync HWDGE ring; everything the
            # post-matmul stages need (bias/ind/masks) issues in parallel on
            # the Scalar HWDGE ring, so the critical hs path is not delayed.
            wt_sb = consts.tile([P, KC, M], dt_mm)
            nc.sync.dma_start(out=wt_sb, in_=wt_r)
            acc_pick = consts.tile([P, NSC], f32)
            acc_ln = consts.tile([Q * 3, NSC], f32)

            hs_sc = []
            for sc in range(NSC):
                hs_q = []
                for q in range(Q):
                    hs_sb = hpool.tile([P, KC, CH], dt_mm, tag="hs")
                    nc.sync.dma_start(
                        out=hs_sb,
                        in_=hsT[sc * Q + q].rearrange("p (k n) -> p k n", k=KC),
                    )
                    hs_q.append(hs_sb)
                hs_sc.append(hs_q)
                if sc == 0:
                    bias_sb = consts.tile([P, 1], f32)
                    nc.scalar.dma_start(out=bias_sb, in_=biasd)
                    biasW_sb = consts.tile([P, 1], f32)
                    nc.scalar.dma_start(out=biasW_sb, in_=biasWd)
                    ind_sb = consts.tile([P, Q * 3], bf16)
                    nc.scalar.dma_start(out=ind_sb, in_=indd)
                    sel_sb = consts.tile([P, NSC * CH], bf16)
                    nc.scalar.dma_start(out=sel_sb, in_=seld)
                    oh3_sb = consts.tile([Q * 3, NSC * CH], bf16)
                    nc.scalar.dma_start(out=oh3_sb, in_=oh3d)

            psg_sc = []
            for sc in range(NSC):
                cs = slice(sc * CH, (sc + 1) * CH)
                hs_q = hs_sc[sc]
                ps = psL.tile([P, CH], f32)
                for kc in range(KC):
                    for q in range(Q):
                        nc.tensor.matmul(
                            ps[32 * q : 32 * (q + 1), :],
                            wt_sb[:, kc, :],
                            hs_q[q][:, kc, :],
                            start=(kc == 0),
                            stop=(kc == KC - 1),
                            tile_position=(0, 32 * q),
                        )

                # sum_b WSCALE*(logits + bias)[sel_b, b] for this superchunk
                junkp = wpool.tile([P, CH], f32, tag="junkp")
                nc.vector.scalar_tensor_tensor(
                    out=junkp,
                    in0=ps,
                    scalar=biasW_sb[:, :],
                    in1=sel_sb[:, cs],
                    op0=add,
                    op1=mult,
                    accum_out=acc_pick[:, sc : sc + 1],
                )

                # exp(logits + bias), cast to bf16 for the fast indicator matmul
                ex = wpool.tile([P, CH], bf16, tag="exp")
                nc.scalar.activation(
                    out=ex, in_=ps, func=Exp, bias=bias_sb[:, :], scale=1.0 / WSCALE
                )

                # per-(chunk, group) sumexp: [128 -> 12]
                psg = psG.tile([Q * 3, CH], f32)
                nc.tensor.matmul(psg, ind_sb[:, :], ex, start=True, stop=True)
                psg_sc.append(psg)

            # ln phase batched after all exp phases: the ACT LUT (exp vs ln)
            # reloads cost ~1.3us each, so avoid alternating functions.
            for sc in range(NSC):
                cs = slice(sc * CH, (sc + 1) * CH)
                lnt = wpool.tile([Q * 3, CH], f32, tag="ln")
                nc.scalar.activation(out=lnt, in_=psg_sc[sc], func=Ln)

                junkl = wpool.tile([Q * 3, CH], f32, tag="junkl")
                nc.vector.scalar_tensor_tensor(
                    out=junkl,
                    in0=lnt,
                    scalar=0.0,
                    in1=oh3_sb[:, cs],
                    op0=add,
                    op1=mult,
                    accum_out=acc_ln[:, sc : sc + 1],
                )

            nc.sync.dma_start(out=out_pick, in_=acc_pick)
            nc.sync.dma_start(out=out_ln, in_=acc_ln)

    nc.finalize()
    return nc


def _pack_inputs(hidden_state, w1, b1, w2, b2, w3, b3, groups, labels):
    _, dt_np = _dtypes()
    bf_np = ml_dtypes.bfloat16
    hs = np.asarray(hidden_state, dtype=np.float32)
    Wpad = np.zeros((M, H), dtype=np.float32)
    Wpad[0:L] = np.asarray(w1, dtype=np.float32)
    Wpad[L : 2 * L] = np.asarray(w2, dtype=np.float32)
    Wpad[2 * L : 3 * L] = np.asarray(w3, dtype=np.float32)
    Wt = np.ascontiguousarray((Wpad.T * WSCALE).astype(dt_np))  # [H, M]

    bias1 = np.zeros(M, dtype=np.float32)
    bias1[0:L] = np.asarray(b1, dtype=np.float32)
    bias1[L : 2 * L] = np.asarray(b2, dtype=np.float32)
    bias1[2 * L : 3 * L] = np.asarray(b3, dtype=np.float32)
    bias4 = np.tile(bias1, Q)[:, None].copy()        # [128, 1]
    biasW = (bias4 * WSCALE).astype(np.float32)

    ind1 = np.zeros((M, 3), dtype=bf_np)
    for g in range(3):
        ind1[g * L : (g + 1) * L, g] = 1.0
    ind4 = np.zeros((P, Q * 3), dtype=bf_np)         # block diag
    for q in range(Q):
        ind4[q * M : (q + 1) * M, q * 3 : (q + 1) * 3] = ind1

    groups = np.asarray(groups).astype(np.int64)
    labels = np.asarray(labels).astype(np.int64)
    col = groups * L + labels                        # [B] in [0, 30)

    hs_cast = hs.astype(dt_np)
    in_maps = []
    for c in range(NCORES):
        sl = slice(c * BC, (c + 1) * BC)
        # [BC, H] -> [NCH, CH, KC, P] -> [NCH, P, KC, CH] -> [NCH, P, KC*CH]
        hsp = (
            hs_cast[sl]
            .reshape(NCH, CH, KC, P)
            .transpose(0, 3, 2, 1)
            .reshape(NCH, P, KC * CH)
        )
        # masks in packed layout: b = sc*Q*CH + q*CH + n -> row block q, col sc*CH+n
        colc = col[sl].reshape(NSC, Q, CH)
        gc = groups[sl].reshape(NSC, Q, CH)
        n_idx = np.arange(CH)
        sel4 = np.zeros((P, NSC * CH), dtype=bf_np)
        oh12 = np.zeros((Q * 3, NSC * CH), dtype=bf_np)
        for sc in range(NSC):
            for q in range(Q):
                sel4[q * M + colc[sc, q], sc * CH + n_idx] = 1.0
                oh12[q * 3 + gc[sc, q], sc * CH + n_idx] = 1.0
        in_maps.append(
            {
                "hsT": np.ascontiguousarray(hsp),
                "wt": Wt,
                "bias4": bias4,
                "biasW": biasW,
                "ind4": ind4,
                "sel4": sel4,
                "oh12": oh12,
            }
        )
    return in_maps


def _run(inputs, trace=False, **kw):
    nc = _build_program()
    in_maps = _pack_inputs(**inputs)
    res = bass_utils.run_bass_kernel_spmd(
        nc, in_maps, list(range(NCORES)), trace=trace, **kw
    )
    total_ln = 0.0
    total_pick = 0.0
    for out in res.results:
        total_ln += float(np.sum(out["out_ln"].astype(np.float64)))
        total_pick += float(np.sum(out["out_pick"].astype(np.float64)))
    loss = (total_ln - total_pick / WSCALE) / B
    return np.float32(loss), res


def kernel(**inputs) -> np.ndarray:
    out, _ = _run(inputs, trace=False)
    return out


def benchmark(inputs, trace=True, **kw):
    """Returns (loss, BassKernelResults) with profiling enabled."""
    return _run(inputs, trace=trace, **kw)
